# revision 6
# baseline (speedup 1.0000x reference)
"""Trainium2 Bass kernel for nn_CombinedLoss_781684048617 (V2).

Pure data parallel over 8 NeuronCores (32768 rows each); only ~100KB of
partial sums leave each core.  All row contractions run on the PE as two
gram matrices against the one-hot y_true logit columns, accumulated in
fp32 PSUM over every 128-row block k:

  psA += yt_k^T @ [w1 | w2' | mm | lse | 1]   (120 x 126)
  psB += yt_k^T @ yp_logits_k                 (120 x 80)

where per element d = yp_par - yt_par:
  w1 = relu(d-1), w2' = min(d+1, 0), mm = min(d^2, 1)
  SmoothL1 sum = sum w1 - sum w2' + 0.5 sum mm   (w = relu(|d|-1) = w1 - w2')

Both inputs stream HBM->SBUF via gpsimd (SWDGE) DMAs casting fp32->fp8e3
in flight (DMA cost is write-charged: fp8 halves it vs fp16).  fp8e3
(e3m4) keeps 4 mantissa bits; y_true one-hot values are exact.  ACT does
exp (fp8 in, fp16 out), ln, and Square; DVE does the softmax-denominator
sum as a tree of strided fp16 adds (2x mode) and the three SmoothL1
columns as tensor_scalar 2-op passes (4x mode).  Final scalar assembly
(divisions, guards, num_params_per_effect table) happens on host in
float64; reg_unmasked is dead code since num_params_per_effect >= 1
implies param count >= mask count.
"""

import sys

import numpy as np

if "/opt/trn_rl_repo" not in sys.path:
    sys.path.insert(0, "/opt/trn_rl_repo")

# ---- problem constants (hardcoded per contract) ----
B_FULL = 262144
NCORES = 8
N_CORE = B_FULL // NCORES  # 32768
E, C, P, ITEM = 5, 16, 8, 24
D = E * ITEM  # 120
LS = 0.05
REG_W = 1.0

PARTS = 128
ROWS_PER_PART = N_CORE // PARTS  # 256
# chunk sizes (rows per partition per chunk); smaller edges shorten
# pipeline fill/drain
CHUNKS = [16, 32, 48, 48, 48, 48, 16]
assert sum(CHUNKS) == ROWS_PER_PART

# psA column layout
NW = E * P  # 40
COL_W1 = 0
COL_W2 = NW
COL_MM = 2 * NW
COL_LSE = 3 * NW
COL_ONE = 3 * NW + E
AW = 3 * NW + E + 1  # 126
BW = E * C  # 80 yp-logit gram columns
GW = AW + BW  # 206

_CACHE = {}


def _build_bass():
    from contextlib import ExitStack

    import concourse.bacc as bacc
    import concourse.bass as bass
    import concourse.tile as tile
    from concourse import mybir

    f32 = mybir.dt.float32
    f16 = mybir.dt.float16
    f8 = mybir.dt.float8e3  # e3m4: 4 mantissa bits, range +-15.9
    AF = mybir.ActivationFunctionType
    OP = mybir.AluOpType

    nc = bacc.Bacc(None, target_bir_lowering=False)
    yp_d = nc.dram_tensor("y_pred", [N_CORE, D], f32, kind="ExternalInput")
    yt_d = nc.dram_tensor("y_true", [N_CORE, D], f32, kind="ExternalInput")
    out_ab = nc.dram_tensor("out_ab", [D, GW], f32, kind="ExternalOutput")

    with tile.TileContext(nc) as tc, ExitStack() as ctx:
        inp = ctx.enter_context(tc.tile_pool(name="inp", bufs=4))
        work = ctx.enter_context(tc.tile_pool(name="work", bufs=3))
        singles = ctx.enter_context(tc.tile_pool(name="singles", bufs=1))
        psum = ctx.enter_context(
            tc.tile_pool(name="psum", bufs=1, space=bass.MemorySpace.PSUM)
        )

        psA = psum.tile([D, AW], f32)
        psB = psum.tile([D, BW], f32)

        row0 = 0
        nchunks = len(CHUNKS)
        for ci, KT in enumerate(CHUNKS):
            ypv = yp_d[row0 : row0 + PARTS * KT].rearrange(
                "(p k) (e i) -> p k e i", k=KT, i=ITEM
            )
            ytv = yt_d[row0 : row0 + PARTS * KT].rearrange(
                "(p k) (e i) -> p k e i", k=KT, i=ITEM
            )
            row0 += PARTS * KT

            yp8 = inp.tile([PARTS, KT, E, ITEM], f8)
            yt8 = inp.tile([PARTS, KT, E, ITEM], f8)
            nc.gpsimd.dma_start(out=yp8, in_=ypv)
            nc.gpsimd.dma_start(out=yt8, in_=ytv)

            M2 = work.tile([PARTS, KT, AW], f16)
            nc.vector.memset(M2[:, :, COL_ONE : COL_ONE + 1], 1.0)

            # ---- smooth l1 columns (latency-first order on DVE) ----
            # d = yp - yt split DVE/Pool: fp8 tensor_tensor runs 1x on DVE,
            # so the idle gpsimd cores absorb ~60% of it
            dd = work.tile([PARTS, KT, E, P], f16)
            KS = max(1, int(round(KT * 0.4)))  # DVE share
            nc.vector.tensor_sub(
                dd[:, 0:KS], yp8[:, 0:KS, :, C:ITEM], yt8[:, 0:KS, :, C:ITEM]
            )
            nc.gpsimd.tensor_sub(
                dd[:, KS:KT], yp8[:, KS:KT, :, C:ITEM], yt8[:, KS:KT, :, C:ITEM]
            )
            w1v = M2[:, :, COL_W1 : COL_W1 + NW].rearrange(
                "p k (e j) -> p k e j", j=P
            )
            nc.vector.tensor_scalar(
                out=w1v, in0=dd, scalar1=1.0, scalar2=0.0,
                op0=OP.subtract, op1=OP.max,
            )
            w2v = M2[:, :, COL_W2 : COL_W2 + NW].rearrange(
                "p k (e j) -> p k e j", j=P
            )
            nc.vector.tensor_scalar(
                out=w2v, in0=dd, scalar1=1.0, scalar2=0.0,
                op0=OP.add, op1=OP.min,
            )

            # ---- cross entropy: lse = ln(sum_c exp(z)) ----
            ex = work.tile([PARTS, KT, E, C], f16)
            nc.scalar.activation(out=ex, in_=yp8[:, :, :, 0:C], func=AF.Exp)
            # d^2 split ACT/DVE (~60/40) to balance the two engines
            vv = work.tile([PARTS, KT, E, P], f16)
            KQ = max(1, int(round(KT * 0.6)))  # ACT share
            nc.scalar.activation(out=vv[:, 0:KQ], in_=dd[:, 0:KQ], func=AF.Square)
            nc.vector.tensor_mul(vv[:, KQ:KT], dd[:, KQ:KT], dd[:, KQ:KT])
            h1 = work.tile([PARTS, KT, E, 8], f16)
            nc.vector.tensor_tensor(h1, ex[:, :, :, 0:8], ex[:, :, :, 8:16], OP.add)
            h2 = work.tile([PARTS, KT, E, 4], f16)
            nc.vector.tensor_tensor(h2, h1[:, :, :, 0:4], h1[:, :, :, 4:8], OP.add)
            h3 = work.tile([PARTS, KT, E, 2], f16)
            nc.vector.tensor_tensor(h3, h2[:, :, :, 0:2], h2[:, :, :, 2:4], OP.add)
            s = work.tile([PARTS, KT, E], f16)
            nc.vector.tensor_tensor(s, h3[:, :, :, 0], h3[:, :, :, 1], OP.add)
            mmv = M2[:, :, COL_MM : COL_MM + NW].rearrange(
                "p k (e j) -> p k e j", j=P
            )
            nc.vector.tensor_scalar(
                out=mmv, in0=vv, scalar1=1.0, scalar2=None, op0=OP.min
            )
            nc.scalar.activation(
                out=M2[:, :, COL_LSE : COL_LSE + E], in_=s, func=AF.Ln
            )

            # ---- gram accumulation on PE (mm2 first: needs only the DMAs) ----
            ytf = yt8.rearrange("p k e i -> p k (e i)")
            for k in range(KT):
                first = ci == 0 and k == 0
                last = ci == nchunks - 1 and k == KT - 1
                nc.tensor.matmul(
                    psB, ytf[:, k], yp8[:, k, :, 0:C], start=first, stop=last
                )
            for k in range(KT):
                first = ci == 0 and k == 0
                last = ci == nchunks - 1 and k == KT - 1
                nc.tensor.matmul(
                    psA, ytf[:, k], M2[:, k], start=first, stop=last
                )

        stage = singles.tile([D, GW], f32)
        nc.scalar.copy(stage[:, 0:AW], psA)
        nc.scalar.copy(stage[:, AW:GW], psB)
        nc.sync.dma_start(out=out_ab[:], in_=stage)

    # Pre-load the one ACT table set covering Exp/Ln/Square/Copy
    # (natural_log_exp_and_others); avoids per-activation table thrash.
    from concourse.hw_specs import get_activation_tables

    tables = list(get_activation_tables(nc.m.arch).items())
    set_id = next(
        i for i, (name, _) in enumerate(tables)
        if name == "natural_log_exp_and_others"
    )
    load = mybir.InstLoadActFuncSet(
        name=nc.get_next_instruction_name(), act_func_set_id=set_id, ins=[], outs=[]
    )
    load.engine = mybir.EngineType.Activation
    nc.register_instruction(load)
    placed = False
    for blk in nc.m.functions[0].blocks:
        for idx, inst in enumerate(blk.instructions):
            if isinstance(inst, mybir.InstActivation):
                blk.instructions.insert(idx, load)
                placed = True
                break
        if placed:
            break
    assert placed

    nc.compile()
    return nc


def _get_nc():
    if "nc" not in _CACHE:
        _CACHE["nc"] = _build_bass()
    return _CACHE["nc"]


def kernel(y_pred, y_true, num_params_per_effect):
    from concourse.bass_utils import run_bass_kernel_spmd

    yp = np.ascontiguousarray(np.asarray(y_pred, dtype=np.float32))
    yt = np.ascontiguousarray(np.asarray(y_true, dtype=np.float32))
    npf = np.asarray(num_params_per_effect, dtype=np.int64)

    yp_sh = yp.reshape(NCORES, N_CORE, D)
    yt_sh = yt.reshape(NCORES, N_CORE, D)
    in_maps = [{"y_pred": yp_sh[i], "y_true": yt_sh[i]} for i in range(NCORES)]

    nc = _get_nc()
    results = run_bass_kernel_spmd(nc, in_maps, list(range(NCORES))).results

    # ---- host-side scalar assembly in float64 ----
    G = np.zeros((D, GW), np.float64)
    for res in results:
        G += np.asarray(res["out_ab"], np.float64)
    GA = G[:, 0:AW]
    GB = G[:, AW:GW]

    Tmask = (np.arange(P)[None, :] < npf[:, None]).astype(np.float64)  # [C,P]
    MSUM = 0.0
    PCNT = 0.0
    LSEt = 0.0
    DX = 0.0
    AFSX = 0.0
    RSUM = 0.0
    for e in range(E):
        rows = slice(ITEM * e, ITEM * e + C)  # yt logit rows of slot e
        cnt = GA[rows, COL_ONE]  # per-class active counts [C]
        MSUM += cnt.sum()
        PCNT += (npf * cnt).sum()
        LSEt += GA[rows, COL_LSE + e].sum()
        DX += np.trace(GB[rows, C * e : C * e + C])
        AFSX += GB[rows, C * e : C * e + C].sum()
        cols = slice(P * e, P * (e + 1))
        sl1 = (
            GA[rows, COL_W1 + P * e : COL_W1 + P * (e + 1)]
            - GA[rows, COL_W2 + P * e : COL_W2 + P * (e + 1)]
            + 0.5 * GA[rows, COL_MM + P * e : COL_MM + P * (e + 1)]
        )
        RSUM += (Tmask * sl1).sum()

    CSUM = LSEt - (1.0 - LS) * DX - (LS / C) * AFSX

    loss_cls = CSUM / max(MSUM, 1.0) if MSUM > 0 else 0.0
    # num_params_per_effect >= 1 guarantees PCNT >= MSUM, so the
    # reference's unmasked-reg fallback (psum==0 while msum>0) is dead.
    loss_reg = (RSUM / max(PCNT, 1.0) if PCNT > 0 else 0.0) if MSUM > 0 else 0.0
    total = loss_cls + REG_W * loss_reg

    return (
        np.float32(total),
        np.float32(loss_cls),
        np.float32(loss_reg),
    )


# revision 7
# speedup vs baseline: 1.0327x; 1.0327x over previous
"""Trainium2 Bass kernel for nn_CombinedLoss_781684048617 (V2).

Pure data parallel over 8 NeuronCores (32768 rows each); only ~100KB of
partial sums leave each core.  All row contractions run on the PE as two
gram matrices against the one-hot y_true logit columns, accumulated in
fp32 PSUM over every 128-row block k:

  psA += yt_k^T @ [w1 | w2' | mm | lse | 1]   (120 x 126)
  psB += yt_k^T @ yp_logits_k                 (120 x 80)

where per element d = yp_par - yt_par:
  w1 = relu(d-1), w2' = min(d+1, 0), mm = min(d^2, 1)
  SmoothL1 sum = sum w1 - sum w2' + 0.5 sum mm   (w = relu(|d|-1) = w1 - w2')

Both inputs stream HBM->SBUF via gpsimd (SWDGE) DMAs casting fp32->fp8e3
in flight (DMA cost is write-charged: fp8 halves it vs fp16).  fp8e3
(e3m4) keeps 4 mantissa bits; y_true one-hot values are exact.  ACT does
exp (fp8 in, fp16 out), ln, and Square; DVE does the softmax-denominator
sum as a tree of strided fp16 adds (2x mode) and the three SmoothL1
columns as tensor_scalar 2-op passes (4x mode).  Final scalar assembly
(divisions, guards, num_params_per_effect table) happens on host in
float64; reg_unmasked is dead code since num_params_per_effect >= 1
implies param count >= mask count.
"""

import sys

import numpy as np

if "/opt/trn_rl_repo" not in sys.path:
    sys.path.insert(0, "/opt/trn_rl_repo")

# ---- problem constants (hardcoded per contract) ----
B_FULL = 262144
NCORES = 8
N_CORE = B_FULL // NCORES  # 32768
E, C, P, ITEM = 5, 16, 8, 24
D = E * ITEM  # 120
LS = 0.05
REG_W = 1.0

PARTS = 128
ROWS_PER_PART = N_CORE // PARTS  # 256
# chunk sizes (rows per partition per chunk); smaller edges shorten
# pipeline fill/drain
CHUNKS = [16, 32, 48, 48, 48, 48, 16]
assert sum(CHUNKS) == ROWS_PER_PART

# psA column layout
NW = E * P  # 40
COL_W1 = 0
COL_W2 = NW
COL_MM = 2 * NW
COL_LSE = 3 * NW
COL_ONE = 3 * NW + E
AW = 3 * NW + E + 1  # 126
BW = E * C  # 80 yp-logit gram columns
GW = AW + BW  # 206

_CACHE = {}


def _build_bass():
    from contextlib import ExitStack

    import concourse.bacc as bacc
    import concourse.bass as bass
    import concourse.tile as tile
    from concourse import mybir

    f32 = mybir.dt.float32
    f16 = mybir.dt.float16
    f8 = mybir.dt.float8e3  # e3m4: 4 mantissa bits, range +-15.9
    AF = mybir.ActivationFunctionType
    OP = mybir.AluOpType

    nc = bacc.Bacc(None, target_bir_lowering=False)
    yp_d = nc.dram_tensor("y_pred", [N_CORE, D], f32, kind="ExternalInput")
    yt_d = nc.dram_tensor("y_true", [N_CORE, D], f32, kind="ExternalInput")
    out_ab = nc.dram_tensor("out_ab", [D, GW], f32, kind="ExternalOutput")

    with tile.TileContext(nc) as tc, ExitStack() as ctx:
        inp = ctx.enter_context(tc.tile_pool(name="inp", bufs=4))
        work = ctx.enter_context(tc.tile_pool(name="work", bufs=3))
        singles = ctx.enter_context(tc.tile_pool(name="singles", bufs=1))
        psum = ctx.enter_context(
            tc.tile_pool(name="psum", bufs=1, space=bass.MemorySpace.PSUM)
        )

        psA = psum.tile([D, AW], f32)
        psB = psum.tile([D, BW], f32)

        row0 = 0
        nchunks = len(CHUNKS)
        for ci, KT in enumerate(CHUNKS):
            ypv = yp_d[row0 : row0 + PARTS * KT].rearrange(
                "(p k) (e i) -> p k e i", k=KT, i=ITEM
            )
            ytv = yt_d[row0 : row0 + PARTS * KT].rearrange(
                "(p k) (e i) -> p k e i", k=KT, i=ITEM
            )
            row0 += PARTS * KT

            yp8 = inp.tile([PARTS, KT, E, ITEM], f8)
            yt8 = inp.tile([PARTS, KT, E, ITEM], f8)
            nc.gpsimd.dma_start(out=yp8, in_=ypv)
            nc.gpsimd.dma_start(out=yt8, in_=ytv)

            M2 = work.tile([PARTS, KT, AW], f16)
            nc.vector.memset(M2[:, :, COL_ONE : COL_ONE + 1], 1.0)

            # ---- smooth l1 columns ----
            # d = yp - yt split DVE/Pool (fp8 tensor_tensor is 1x on DVE, so
            # the idle gpsimd cores absorb half); separate tiles + per-half
            # column passes keep the fast half independent of the slow one
            KS = max(1, int(round(KT * 0.5)))  # DVE share
            dda = work.tile([PARTS, KS, E, P], f16)
            ddb = work.tile([PARTS, KT - KS, E, P], f16)
            nc.vector.tensor_sub(
                dda, yp8[:, 0:KS, :, C:ITEM], yt8[:, 0:KS, :, C:ITEM]
            )
            nc.gpsimd.tensor_sub(
                ddb, yp8[:, KS:KT, :, C:ITEM], yt8[:, KS:KT, :, C:ITEM]
            )
            w1v = M2[:, :, COL_W1 : COL_W1 + NW].rearrange(
                "p k (e j) -> p k e j", j=P
            )
            w2v = M2[:, :, COL_W2 : COL_W2 + NW].rearrange(
                "p k (e j) -> p k e j", j=P
            )
            mmv = M2[:, :, COL_MM : COL_MM + NW].rearrange(
                "p k (e j) -> p k e j", j=P
            )
            vva = work.tile([PARTS, KS, E, P], f16)
            vvb = work.tile([PARTS, KT - KS, E, P], f16)
            # a-half (feeds from DVE sub; square on ACT)
            nc.vector.tensor_scalar(
                out=w1v[:, 0:KS], in0=dda, scalar1=1.0, scalar2=0.0,
                op0=OP.subtract, op1=OP.max,
            )
            nc.vector.tensor_scalar(
                out=w2v[:, 0:KS], in0=dda, scalar1=1.0, scalar2=0.0,
                op0=OP.add, op1=OP.min,
            )
            nc.scalar.activation(out=vva, in_=dda, func=AF.Square)
            nc.vector.tensor_scalar(
                out=mmv[:, 0:KS], in0=vva, scalar1=1.0, scalar2=None, op0=OP.min
            )

            # ---- cross entropy: lse = ln(sum_c exp(z)) ----
            ex = work.tile([PARTS, KT, E, C], f16)
            nc.scalar.activation(out=ex, in_=yp8[:, :, :, 0:C], func=AF.Exp)
            h1 = work.tile([PARTS, KT, E, 8], f16)
            nc.vector.tensor_tensor(h1, ex[:, :, :, 0:8], ex[:, :, :, 8:16], OP.add)
            h2 = work.tile([PARTS, KT, E, 4], f16)
            nc.vector.tensor_tensor(h2, h1[:, :, :, 0:4], h1[:, :, :, 4:8], OP.add)
            # b-half (feeds from Pool sub; square on DVE)
            nc.vector.tensor_scalar(
                out=w1v[:, KS:KT], in0=ddb, scalar1=1.0, scalar2=0.0,
                op0=OP.subtract, op1=OP.max,
            )
            nc.vector.tensor_scalar(
                out=w2v[:, KS:KT], in0=ddb, scalar1=1.0, scalar2=0.0,
                op0=OP.add, op1=OP.min,
            )
            nc.vector.tensor_mul(vvb, ddb, ddb)
            nc.vector.tensor_scalar(
                out=mmv[:, KS:KT], in0=vvb, scalar1=1.0, scalar2=None, op0=OP.min
            )
            h3 = work.tile([PARTS, KT, E, 2], f16)
            nc.vector.tensor_tensor(h3, h2[:, :, :, 0:2], h2[:, :, :, 2:4], OP.add)
            s = work.tile([PARTS, KT, E], f16)
            nc.vector.tensor_tensor(s, h3[:, :, :, 0], h3[:, :, :, 1], OP.add)
            nc.scalar.activation(
                out=M2[:, :, COL_LSE : COL_LSE + E], in_=s, func=AF.Ln
            )

            # ---- gram accumulation on PE (mm2 first: needs only the DMAs) ----
            ytf = yt8.rearrange("p k e i -> p k (e i)")
            for k in range(KT):
                first = ci == 0 and k == 0
                last = ci == nchunks - 1 and k == KT - 1
                nc.tensor.matmul(
                    psB, ytf[:, k], yp8[:, k, :, 0:C], start=first, stop=last
                )
            for k in range(KT):
                first = ci == 0 and k == 0
                last = ci == nchunks - 1 and k == KT - 1
                nc.tensor.matmul(
                    psA, ytf[:, k], M2[:, k], start=first, stop=last
                )

        stage = singles.tile([D, GW], f32)
        nc.scalar.copy(stage[:, 0:AW], psA)
        nc.scalar.copy(stage[:, AW:GW], psB)
        nc.sync.dma_start(out=out_ab[:], in_=stage)

    # Pre-load the one ACT table set covering Exp/Ln/Square/Copy
    # (natural_log_exp_and_others); avoids per-activation table thrash.
    from concourse.hw_specs import get_activation_tables

    tables = list(get_activation_tables(nc.m.arch).items())
    set_id = next(
        i for i, (name, _) in enumerate(tables)
        if name == "natural_log_exp_and_others"
    )
    load = mybir.InstLoadActFuncSet(
        name=nc.get_next_instruction_name(), act_func_set_id=set_id, ins=[], outs=[]
    )
    load.engine = mybir.EngineType.Activation
    nc.register_instruction(load)
    placed = False
    for blk in nc.m.functions[0].blocks:
        for idx, inst in enumerate(blk.instructions):
            if isinstance(inst, mybir.InstActivation):
                blk.instructions.insert(idx, load)
                placed = True
                break
        if placed:
            break
    assert placed

    nc.compile()
    return nc


def _get_nc():
    if "nc" not in _CACHE:
        _CACHE["nc"] = _build_bass()
    return _CACHE["nc"]


def kernel(y_pred, y_true, num_params_per_effect):
    from concourse.bass_utils import run_bass_kernel_spmd

    yp = np.ascontiguousarray(np.asarray(y_pred, dtype=np.float32))
    yt = np.ascontiguousarray(np.asarray(y_true, dtype=np.float32))
    npf = np.asarray(num_params_per_effect, dtype=np.int64)

    yp_sh = yp.reshape(NCORES, N_CORE, D)
    yt_sh = yt.reshape(NCORES, N_CORE, D)
    in_maps = [{"y_pred": yp_sh[i], "y_true": yt_sh[i]} for i in range(NCORES)]

    nc = _get_nc()
    results = run_bass_kernel_spmd(nc, in_maps, list(range(NCORES))).results

    # ---- host-side scalar assembly in float64 ----
    G = np.zeros((D, GW), np.float64)
    for res in results:
        G += np.asarray(res["out_ab"], np.float64)
    GA = G[:, 0:AW]
    GB = G[:, AW:GW]

    Tmask = (np.arange(P)[None, :] < npf[:, None]).astype(np.float64)  # [C,P]
    MSUM = 0.0
    PCNT = 0.0
    LSEt = 0.0
    DX = 0.0
    AFSX = 0.0
    RSUM = 0.0
    for e in range(E):
        rows = slice(ITEM * e, ITEM * e + C)  # yt logit rows of slot e
        cnt = GA[rows, COL_ONE]  # per-class active counts [C]
        MSUM += cnt.sum()
        PCNT += (npf * cnt).sum()
        LSEt += GA[rows, COL_LSE + e].sum()
        DX += np.trace(GB[rows, C * e : C * e + C])
        AFSX += GB[rows, C * e : C * e + C].sum()
        cols = slice(P * e, P * (e + 1))
        sl1 = (
            GA[rows, COL_W1 + P * e : COL_W1 + P * (e + 1)]
            - GA[rows, COL_W2 + P * e : COL_W2 + P * (e + 1)]
            + 0.5 * GA[rows, COL_MM + P * e : COL_MM + P * (e + 1)]
        )
        RSUM += (Tmask * sl1).sum()

    CSUM = LSEt - (1.0 - LS) * DX - (LS / C) * AFSX

    loss_cls = CSUM / max(MSUM, 1.0) if MSUM > 0 else 0.0
    # num_params_per_effect >= 1 guarantees PCNT >= MSUM, so the
    # reference's unmasked-reg fallback (psum==0 while msum>0) is dead.
    loss_reg = (RSUM / max(PCNT, 1.0) if PCNT > 0 else 0.0) if MSUM > 0 else 0.0
    total = loss_cls + REG_W * loss_reg

    return (
        np.float32(total),
        np.float32(loss_cls),
        np.float32(loss_reg),
    )


# revision 8
# speedup vs baseline: 1.1426x; 1.1064x over previous
"""Trainium2 Bass kernel for nn_CombinedLoss_781684048617 (V2).

Pure data parallel over 8 NeuronCores (32768 rows each); only ~100KB of
partial sums leave each core.  All row contractions run on the PE as two
gram matrices against the one-hot y_true logit columns, accumulated in
fp32 PSUM over every 128-row block k:

  psA += yt_k^T @ [w1 | w2' | mm | lse | 1]   (120 x 126)
  psB += yt_k^T @ yp_logits_k                 (120 x 80)

where per element d = yp_par - yt_par:
  w1 = relu(d-1), w2' = min(d+1, 0), mm = min(d^2, 1)
  SmoothL1 sum = sum w1 - sum w2' + 0.5 sum mm   (w = relu(|d|-1) = w1 - w2')

Both inputs stream HBM->SBUF via gpsimd (SWDGE) DMAs casting fp32->fp8e3
in flight (DMA cost is write-charged: fp8 halves it vs fp16).  fp8e3
(e3m4) keeps 4 mantissa bits; y_true one-hot values are exact.  ACT does
exp (fp8 in, fp16 out), ln, and Square; DVE does the softmax-denominator
sum as a tree of strided fp16 adds (2x mode) and the three SmoothL1
columns as tensor_scalar 2-op passes (4x mode).  Final scalar assembly
(divisions, guards, num_params_per_effect table) happens on host in
float64; reg_unmasked is dead code since num_params_per_effect >= 1
implies param count >= mask count.
"""

import sys

import numpy as np

if "/opt/trn_rl_repo" not in sys.path:
    sys.path.insert(0, "/opt/trn_rl_repo")

# ---- problem constants (hardcoded per contract) ----
B_FULL = 262144
NCORES = 8
N_CORE = B_FULL // NCORES  # 32768
E, C, P, ITEM = 5, 16, 8, 24
D = E * ITEM  # 120
LS = 0.05
REG_W = 1.0

PARTS = 128
ROWS_PER_PART = N_CORE // PARTS  # 256
# chunk sizes (rows per partition per chunk); smaller edges shorten
# pipeline fill/drain
CHUNKS = [16, 32, 48, 48, 48, 48, 16]
assert sum(CHUNKS) == ROWS_PER_PART

# psA column layout
NW = E * P  # 40
COL_W1 = 0
COL_W2 = NW
COL_MM = 2 * NW
COL_LSE = 3 * NW
COL_ONE = 3 * NW + E
AW = 3 * NW + E + 1  # 126
BW = E * C  # 80 yp-logit gram columns
GW = AW + BW  # 206

_CACHE = {}


def _build_bass():
    from contextlib import ExitStack

    import concourse.bacc as bacc
    import concourse.bass as bass
    import concourse.tile as tile
    from concourse import mybir

    f32 = mybir.dt.float32
    f16 = mybir.dt.float16
    f8 = mybir.dt.float8e3  # e3m4: 4 mantissa bits, range +-15.9
    AF = mybir.ActivationFunctionType
    OP = mybir.AluOpType

    nc = bacc.Bacc(None, target_bir_lowering=False)
    yp_d = nc.dram_tensor("y_pred", [N_CORE, D], f32, kind="ExternalInput")
    yt_d = nc.dram_tensor("y_true", [N_CORE, D], f32, kind="ExternalInput")
    out_ab = nc.dram_tensor("out_ab", [D, GW], f32, kind="ExternalOutput")

    with tile.TileContext(nc) as tc, ExitStack() as ctx:
        inp = ctx.enter_context(tc.tile_pool(name="inp", bufs=4))
        work = ctx.enter_context(tc.tile_pool(name="work", bufs=3))
        singles = ctx.enter_context(tc.tile_pool(name="singles", bufs=1))
        psum = ctx.enter_context(
            tc.tile_pool(name="psum", bufs=1, space=bass.MemorySpace.PSUM)
        )

        psA = psum.tile([D, AW], f32)
        psB = psum.tile([D, BW], f32)

        nchunks = len(CHUNKS)
        # prefetch: issue chunk c+1's DMAs before chunk c's Pool compute so
        # SWDGE descriptor gen isn't queued behind the gpsimd sub
        row_starts = []
        r = 0
        for KT in CHUNKS:
            row_starts.append(r)
            r += PARTS * KT

        def issue_dmas(ci):
            KT = CHUNKS[ci]
            r0 = row_starts[ci]
            ypv = yp_d[r0 : r0 + PARTS * KT].rearrange(
                "(p k) (e i) -> p k e i", k=KT, i=ITEM
            )
            ytv = yt_d[r0 : r0 + PARTS * KT].rearrange(
                "(p k) (e i) -> p k e i", k=KT, i=ITEM
            )
            yp8 = inp.tile([PARTS, KT, E, ITEM], f8)
            yt8 = inp.tile([PARTS, KT, E, ITEM], f8)
            nc.gpsimd.dma_start(out=yp8, in_=ypv)
            nc.gpsimd.dma_start(out=yt8, in_=ytv)
            return yp8, yt8

        pending = issue_dmas(0)
        for ci, KT in enumerate(CHUNKS):
            yp8, yt8 = pending
            if ci + 1 < nchunks:
                pending = issue_dmas(ci + 1)

            M2 = work.tile([PARTS, KT, AW], f16)
            nc.vector.memset(M2[:, :, COL_ONE : COL_ONE + 1], 1.0)

            # ---- smooth l1 columns ----
            # d = yp - yt split DVE/Pool (fp8 tensor_tensor is 1x on DVE, so
            # the idle gpsimd cores absorb half); separate tiles + per-half
            # column passes keep the fast half independent of the slow one
            KS = max(1, int(round(KT * 0.5)))  # DVE share
            dda = work.tile([PARTS, KS, E, P], f16)
            ddb = work.tile([PARTS, KT - KS, E, P], f16)
            nc.vector.tensor_sub(
                dda, yp8[:, 0:KS, :, C:ITEM], yt8[:, 0:KS, :, C:ITEM]
            )
            nc.gpsimd.tensor_sub(
                ddb, yp8[:, KS:KT, :, C:ITEM], yt8[:, KS:KT, :, C:ITEM]
            )
            w1v = M2[:, :, COL_W1 : COL_W1 + NW].rearrange(
                "p k (e j) -> p k e j", j=P
            )
            w2v = M2[:, :, COL_W2 : COL_W2 + NW].rearrange(
                "p k (e j) -> p k e j", j=P
            )
            mmv = M2[:, :, COL_MM : COL_MM + NW].rearrange(
                "p k (e j) -> p k e j", j=P
            )
            vva = work.tile([PARTS, KS, E, P], f16)
            vvb = work.tile([PARTS, KT - KS, E, P], f16)
            # a-half (feeds from DVE sub; square on ACT)
            nc.vector.tensor_scalar(
                out=w1v[:, 0:KS], in0=dda, scalar1=1.0, scalar2=0.0,
                op0=OP.subtract, op1=OP.max,
            )
            nc.vector.tensor_scalar(
                out=w2v[:, 0:KS], in0=dda, scalar1=1.0, scalar2=0.0,
                op0=OP.add, op1=OP.min,
            )
            nc.scalar.activation(out=vva, in_=dda, func=AF.Square)
            nc.vector.tensor_scalar(
                out=mmv[:, 0:KS], in0=vva, scalar1=1.0, scalar2=None, op0=OP.min
            )

            # ---- cross entropy: lse = ln(sum_c exp(z)) ----
            ex = work.tile([PARTS, KT, E, C], f16)
            nc.scalar.activation(out=ex, in_=yp8[:, :, :, 0:C], func=AF.Exp)
            h1 = work.tile([PARTS, KT, E, 8], f16)
            nc.vector.tensor_tensor(h1, ex[:, :, :, 0:8], ex[:, :, :, 8:16], OP.add)
            h2 = work.tile([PARTS, KT, E, 4], f16)
            nc.vector.tensor_tensor(h2, h1[:, :, :, 0:4], h1[:, :, :, 4:8], OP.add)
            # b-half (feeds from Pool sub; square on DVE)
            nc.vector.tensor_scalar(
                out=w1v[:, KS:KT], in0=ddb, scalar1=1.0, scalar2=0.0,
                op0=OP.subtract, op1=OP.max,
            )
            nc.vector.tensor_scalar(
                out=w2v[:, KS:KT], in0=ddb, scalar1=1.0, scalar2=0.0,
                op0=OP.add, op1=OP.min,
            )
            nc.vector.tensor_mul(vvb, ddb, ddb)
            nc.vector.tensor_scalar(
                out=mmv[:, KS:KT], in0=vvb, scalar1=1.0, scalar2=None, op0=OP.min
            )
            h3 = work.tile([PARTS, KT, E, 2], f16)
            nc.vector.tensor_tensor(h3, h2[:, :, :, 0:2], h2[:, :, :, 2:4], OP.add)
            s = work.tile([PARTS, KT, E], f16)
            nc.vector.tensor_tensor(s, h3[:, :, :, 0], h3[:, :, :, 1], OP.add)
            nc.scalar.activation(
                out=M2[:, :, COL_LSE : COL_LSE + E], in_=s, func=AF.Ln
            )

            # ---- gram accumulation on PE (mm2 first: needs only the DMAs) ----
            ytf = yt8.rearrange("p k e i -> p k (e i)")
            for k in range(KT):
                first = ci == 0 and k == 0
                last = ci == nchunks - 1 and k == KT - 1
                nc.tensor.matmul(
                    psB, ytf[:, k], yp8[:, k, :, 0:C], start=first, stop=last
                )
            for k in range(KT):
                first = ci == 0 and k == 0
                last = ci == nchunks - 1 and k == KT - 1
                nc.tensor.matmul(
                    psA, ytf[:, k], M2[:, k], start=first, stop=last
                )

        stage = singles.tile([D, GW], f32)
        nc.scalar.copy(stage[:, 0:AW], psA)
        nc.scalar.copy(stage[:, AW:GW], psB)
        nc.sync.dma_start(out=out_ab[:], in_=stage)

    # Pre-load the one ACT table set covering Exp/Ln/Square/Copy
    # (natural_log_exp_and_others); avoids per-activation table thrash.
    from concourse.hw_specs import get_activation_tables

    tables = list(get_activation_tables(nc.m.arch).items())
    set_id = next(
        i for i, (name, _) in enumerate(tables)
        if name == "natural_log_exp_and_others"
    )
    load = mybir.InstLoadActFuncSet(
        name=nc.get_next_instruction_name(), act_func_set_id=set_id, ins=[], outs=[]
    )
    load.engine = mybir.EngineType.Activation
    nc.register_instruction(load)
    placed = False
    for blk in nc.m.functions[0].blocks:
        for idx, inst in enumerate(blk.instructions):
            if isinstance(inst, mybir.InstActivation):
                blk.instructions.insert(idx, load)
                placed = True
                break
        if placed:
            break
    assert placed

    nc.compile()
    return nc


def _get_nc():
    if "nc" not in _CACHE:
        _CACHE["nc"] = _build_bass()
    return _CACHE["nc"]


def kernel(y_pred, y_true, num_params_per_effect):
    from concourse.bass_utils import run_bass_kernel_spmd

    yp = np.ascontiguousarray(np.asarray(y_pred, dtype=np.float32))
    yt = np.ascontiguousarray(np.asarray(y_true, dtype=np.float32))
    npf = np.asarray(num_params_per_effect, dtype=np.int64)

    yp_sh = yp.reshape(NCORES, N_CORE, D)
    yt_sh = yt.reshape(NCORES, N_CORE, D)
    in_maps = [{"y_pred": yp_sh[i], "y_true": yt_sh[i]} for i in range(NCORES)]

    nc = _get_nc()
    results = run_bass_kernel_spmd(nc, in_maps, list(range(NCORES))).results

    # ---- host-side scalar assembly in float64 ----
    G = np.zeros((D, GW), np.float64)
    for res in results:
        G += np.asarray(res["out_ab"], np.float64)
    GA = G[:, 0:AW]
    GB = G[:, AW:GW]

    Tmask = (np.arange(P)[None, :] < npf[:, None]).astype(np.float64)  # [C,P]
    MSUM = 0.0
    PCNT = 0.0
    LSEt = 0.0
    DX = 0.0
    AFSX = 0.0
    RSUM = 0.0
    for e in range(E):
        rows = slice(ITEM * e, ITEM * e + C)  # yt logit rows of slot e
        cnt = GA[rows, COL_ONE]  # per-class active counts [C]
        MSUM += cnt.sum()
        PCNT += (npf * cnt).sum()
        LSEt += GA[rows, COL_LSE + e].sum()
        DX += np.trace(GB[rows, C * e : C * e + C])
        AFSX += GB[rows, C * e : C * e + C].sum()
        cols = slice(P * e, P * (e + 1))
        sl1 = (
            GA[rows, COL_W1 + P * e : COL_W1 + P * (e + 1)]
            - GA[rows, COL_W2 + P * e : COL_W2 + P * (e + 1)]
            + 0.5 * GA[rows, COL_MM + P * e : COL_MM + P * (e + 1)]
        )
        RSUM += (Tmask * sl1).sum()

    CSUM = LSEt - (1.0 - LS) * DX - (LS / C) * AFSX

    loss_cls = CSUM / max(MSUM, 1.0) if MSUM > 0 else 0.0
    # num_params_per_effect >= 1 guarantees PCNT >= MSUM, so the
    # reference's unmasked-reg fallback (psum==0 while msum>0) is dead.
    loss_reg = (RSUM / max(PCNT, 1.0) if PCNT > 0 else 0.0) if MSUM > 0 else 0.0
    total = loss_cls + REG_W * loss_reg

    return (
        np.float32(total),
        np.float32(loss_cls),
        np.float32(loss_reg),
    )
